# revision 15
# baseline (speedup 1.0000x reference)
"""LoRA linear (y = x @ (W + s*B@A)^T + bias) on 8 Trainium2 NeuronCores.

Strategy: pure data parallel over the token dim. The LoRA update is folded
into the weight on the host (W' = W + 4.0 * B @ A, rank-8 update), so the
device kernel is a plain linear. x and W' are cast to bf16 on the host
(end-to-end rel fro err ~3e-3, well under the 2e-2 gate); out is written
bf16 and upcast on the host. PSUM accumulation stays fp32.

Per core: out[2048, 1024] = x_shard[2048, 1024] @ w' + bias, built as
256 matmuls: stationary x [128d,128tok] bf16 (FWL), moving w [128d,512o].

Key constraint discovered by tracing: all dma_starts share 8 DMAHW
completion-semaphore slots, and each completion costs the ~2-3.5us HBM
read+receipt latency, so descriptor N+8's issue stalls on descriptor N's
completion. The host therefore pre-swizzles x and w into layouts where
the whole startup working set is EXACTLY 8 descriptors, in consumption
order:
  xR[p, g*4096 + t*512 + nn] = x[g*512+nn, t*128+p]   (g=token group of
      512, t=contraction tile, nn=token within group, p=partition)
  wR[p, t*1024 + o]          = w'[t*128+p, o]
  sync queue:   x(t0,g0) | x(t1-3,g0) | x(t4-7,g0) | x(g1) | x(g2) | x(g3)
  scalar queue: w(t0h0) | w(t0h1,t1) | w(t2-4) | w(t5-7) | bias | 16 stores

Other structure:
  - ~15 warmup matmuls on zeroed bf16 scratch keep the PE busy from the
    engine preamble to first-data (~10.5us) so the HAM clock gate
    (1.2 -> 2.4 GHz after ~3.4us of sustained activity) flips early.
  - group 0 runs d-outer (arrival order); its last d-row goes i-outer
    with immediate per-tile eviction so psum slots free staggered.
  - groups 1-3 run i-outer / d-inner: evictions overlap accumulation.
  - the final tile goes h-outer so only one [128,512] eviction + 128KB
    store remain after the last matmul.
"""

import os
import sys

import numpy as np

for _p in ("/opt/trn_rl_repo", "/opt/pypackages"):
    if os.path.isdir(_p) and _p not in sys.path:
        sys.path.append(_p)

try:
    import jax

    jax.config.update(
        "jax_compilation_cache_dir", os.path.expanduser("~/.cache/jax_bass_cache")
    )
    jax.config.update("jax_persistent_cache_min_compile_time_secs", 0.0)
except Exception:
    pass

try:
    # bass_utils imports this when tracing is requested via BASS_TRACE; the
    # agent image ships a stub antenv without it. Register a no-op fallback
    # so a trace request degrades to "no trace" instead of crashing.
    from antenv import axon_hooks as _axon_hooks  # noqa: F401
except ImportError:
    import types as _types

    import antenv as _antenv

    _hooks = _types.ModuleType("antenv.axon_hooks")
    _hooks._hook = None
    _hooks.set_axon_ntff_profile_hook = lambda h: setattr(_hooks, "_hook", h)
    _hooks.get_axon_ntff_profile_hook = lambda: _hooks._hook
    sys.modules["antenv.axon_hooks"] = _hooks
    _antenv.axon_hooks = _hooks

import ml_dtypes  # noqa: E402

import concourse.bass as bass  # noqa: E402,F401
import concourse.mybir as mybir  # noqa: E402
import concourse.tile as tile  # noqa: E402
from concourse import bacc  # noqa: E402
from concourse.bass_utils import run_bass_kernel_spmd  # noqa: E402

N_CORES = 8
N_TOK, D_IN, D_OUT = 16384, 1024, 1024
N_SHARD = N_TOK // N_CORES  # 2048 tokens per core
P = 128
SCALING = 4.0  # alpha / r = 32 / 8
BF16 = ml_dtypes.bfloat16

KT = D_IN // P  # 8 contraction tiles
NBLK = 512  # tokens per group (4 psum tiles of 128)
GRP = NBLK // P  # 4 psum tiles accumulated concurrently (8 banks)
OH = 512  # one PSUM bank of fp32 = max moving free dim
NGRP = N_SHARD // NBLK
XCOL = KT * NBLK  # 4096 xR columns per token group
N_WARM = 14

_CACHE: dict = {}


def build_nc():
    f32 = mybir.dt.float32
    bf16 = mybir.dt.bfloat16
    nc = bacc.Bacc("TRN2", target_bir_lowering=False, debug=False)

    xR = nc.dram_tensor("xR", [P, NGRP * XCOL], bf16, kind="ExternalInput")
    wR = nc.dram_tensor("wR", [P, KT * D_OUT], bf16, kind="ExternalInput")
    bias = nc.dram_tensor("bias", [1, D_OUT], f32, kind="ExternalInput")
    out = nc.dram_tensor("out", [N_SHARD, D_OUT], bf16, kind="ExternalOutput")

    with tile.TileContext(nc) as tc:
        with tc.tile_pool(name="const", bufs=1) as const_pool, \
                tc.tile_pool(name="op", bufs=6) as out_pool, \
                tc.tile_pool(name="ps", bufs=GRP, space="PSUM") as psum_pool:
            x_all = const_pool.tile([P, NGRP * XCOL], bf16, name="x_all")
            w_all = const_pool.tile([P, KT * D_OUT], bf16, name="w_all")
            bias_sb = const_pool.tile([P, D_OUT], f32)

            def xs(g, t, i):  # stationary slice for (group, d-tile, tok blk)
                c0 = g * XCOL + t * NBLK + i * P
                return x_all[:, c0:c0 + P]

            def ws(t, h):  # moving slice for (d-tile, o-half)
                c0 = t * D_OUT + h * OH
                return w_all[:, c0:c0 + OH]

            # Warmup scratch + matmuls (see module docstring).
            warm = const_pool.tile([P, 256], bf16)
            nc.gpsimd.memset(warm[:], 0.0)
            warm_ps = psum_pool.tile([P, 256], f32, name="warm_ps", tag="psum")
            for _ in range(N_WARM):
                nc.tensor.matmul(warm_ps[:], warm[:, 0:P], warm[:],
                                 start=True, stop=True)

            # Startup descriptors, SMALL and interleaved in strict need
            # order: the SDMA engines round-robin over all in-flight
            # descriptors at packet granularity, so a descriptor's
            # completion time ~ (bytes in flight ahead of it) / BW. Big
            # early descriptors destroy arrival ordering; bulk (groups 1-3
            # of x) goes last as 1MB descriptors on recycled slots.
            def wdma(c0, c1):
                nc.scalar.dma_start(w_all[:, c0:c1], wR[:, c0:c1])

            def xdma(c0, c1):
                nc.sync.dma_start(x_all[:, c0:c1], xR[:, c0:c1])

            # bias takes wave-1 lane #8 on the scalar ring: it ring-drains
            # after w t2 (landing ~+10us, well before the first eviction at
            # ~+16us) without crowding the first w tiles. Issuing it early
            # on the gpsimd SWDGE ring instead steals early SDMA bandwidth
            # and pushes w t0h1 ~2us late (measured).
            wdma(0, OH)                      # w t0 h0
            xdma(0, NBLK)                    # x t0 g0
            wdma(OH, D_OUT)                  # w t0 h1
            xdma(NBLK, 2 * NBLK)             # x t1 g0
            wdma(D_OUT, 2 * D_OUT)           # w t1
            xdma(2 * NBLK, 3 * NBLK)         # x t2 g0
            wdma(2 * D_OUT, 3 * D_OUT)       # w t2
            nc.scalar.dma_start(bias_sb[:], bias[:].to_broadcast((P, D_OUT)))
            xdma(3 * NBLK, 4 * NBLK)         # x t3 g0
            wdma(3 * D_OUT, 4 * D_OUT)       # w t3
            xdma(4 * NBLK, 6 * NBLK)         # x t4-t5 g0
            wdma(4 * D_OUT, 6 * D_OUT)       # w t4-t5
            xdma(6 * NBLK, 8 * NBLK)         # x t6-t7 g0
            wdma(6 * D_OUT, 8 * D_OUT)       # w t6-t7
            for g in range(1, NGRP):
                xdma(g * XCOL, (g + 1) * XCOL)

            def evict(g, i, psum):
                n0 = g * NBLK + i * P
                o_sb = out_pool.tile([P, D_OUT], bf16)
                for h in range(2):
                    sl = slice(h * OH, (h + 1) * OH)
                    nc.vector.tensor_add(o_sb[:, sl], psum[:, sl],
                                         bias_sb[:, sl])
                nc.scalar.dma_start(out[n0:n0 + P, :], o_sb[:])

            # Group 0: d-outer / h-middle / i-inner for t0-t5 (arrival
            # order), then the last TWO d rows go i-outer with immediate
            # per-tile eviction: TT(g0,i0) starts ~3.5us before group end,
            # so all four evictions are done when group 1 reuses the psum
            # slots (a one-row stagger left a ~1us reuse stall).
            psums = [
                psum_pool.tile([P, D_OUT], f32, name=f"ps_g0_{i}", tag="psum")
                for i in range(GRP)
            ]
            for t in range(KT - 2):
                for h in range(2):
                    osl = slice(h * OH, (h + 1) * OH)
                    for i in range(GRP):
                        nc.tensor.matmul(psums[i][:, osl], xs(0, t, i),
                                         ws(t, h), start=(t == 0), stop=False)
            for i in range(GRP):
                for t in (KT - 2, KT - 1):
                    for h in range(2):
                        osl = slice(h * OH, (h + 1) * OH)
                        nc.tensor.matmul(psums[i][:, osl], xs(0, t, i),
                                         ws(t, h), start=False,
                                         stop=(t == KT - 1))
                evict(0, i, psums[i])

            # Groups 1-3: i-outer / d-inner; final tile h-outer.
            for g in range(1, NGRP):
                for i in range(GRP):
                    last = (g == NGRP - 1 and i == GRP - 1)
                    if not last:
                        psum = psum_pool.tile([P, D_OUT], f32,
                                              name=f"ps_g{g}_{i}", tag="psum")
                        for t in range(KT):
                            for h in range(2):
                                osl = slice(h * OH, (h + 1) * OH)
                                nc.tensor.matmul(psum[:, osl], xs(g, t, i),
                                                 ws(t, h), start=(t == 0),
                                                 stop=(t == KT - 1))
                        evict(g, i, psum)
                    else:
                        # Final tile: h-outer on TWO separate one-bank psum
                        # tiles (a single [128,1024] tile makes h1's
                        # start=True wait on h0's eviction — tile-granular
                        # WAR — costing ~1.2us on the critical tail).
                        n0 = g * NBLK + i * P
                        o_sb = out_pool.tile([P, D_OUT], bf16)
                        for h in range(2):
                            ps_h = psum_pool.tile([P, OH], f32,
                                                  name=f"ps_last_h{h}",
                                                  tag="psum")
                            osl = slice(h * OH, (h + 1) * OH)
                            for t in range(KT):
                                nc.tensor.matmul(ps_h[:], xs(g, t, i),
                                                 ws(t, h), start=(t == 0),
                                                 stop=(t == KT - 1))
                            nc.vector.tensor_add(o_sb[:, osl], ps_h[:],
                                                 bias_sb[:, osl])
                            nc.scalar.dma_start(out[n0:n0 + P, osl],
                                                o_sb[:, osl])

    nc.finalize()
    return nc


def _get_nc():
    if "nc" not in _CACHE:
        _CACHE["nc"] = build_nc()
    return _CACHE["nc"]


def _swizzle_x(x_shard):
    # [2048, 1024] -> xR[p, g*4096 + t*512 + nn] = x[g*512+nn, t*128+p]
    v = x_shard.reshape(NGRP, NBLK, KT, P)  # [g, nn, t, p]
    v = v.transpose(3, 0, 2, 1)  # [p, g, t, nn]
    return np.ascontiguousarray(v.reshape(P, NGRP * XCOL))


def kernel(x, weight, bias, A, B):
    x = np.asarray(x, dtype=np.float32)
    weight = np.asarray(weight, dtype=np.float32)
    bias = np.asarray(bias, dtype=np.float32)
    A = np.asarray(A, dtype=np.float32)
    B = np.asarray(B, dtype=np.float32)

    # Fold the rank-8 LoRA update into the weight (exact up to fp32 rounding).
    w_eff = (
        weight.astype(np.float64) + SCALING * (B.astype(np.float64) @ A.astype(np.float64))
    ).astype(np.float32)
    # wR[p, t*1024 + o] = w_eff.T[t*128+p, o] = w_eff[o, t*128+p]
    wR = np.ascontiguousarray(
        w_eff.T.astype(BF16).reshape(KT, P, D_OUT).transpose(1, 0, 2).reshape(
            P, KT * D_OUT)
    )
    xb = x.astype(BF16)
    bias2d = np.ascontiguousarray(bias.reshape(1, D_OUT))

    nc = _get_nc()
    in_maps = [
        {
            "xR": _swizzle_x(xb[c * N_SHARD:(c + 1) * N_SHARD]),
            "wR": wR,
            "bias": bias2d,
        }
        for c in range(N_CORES)
    ]
    trace_kwargs = {}
    if os.environ.get("KERNEL_TRACE") == "1":
        trace_kwargs = {"trace": True}
    res = run_bass_kernel_spmd(nc, in_maps, list(range(N_CORES)), **trace_kwargs)
    _CACHE["last_results"] = res
    out = np.concatenate([r["out"] for r in res.results], axis=0)
    return out.astype(np.float32)


# revision 20
# speedup vs baseline: 1.0473x; 1.0473x over previous
"""LoRA linear (y = x @ (W + s*B@A)^T + bias) on 8 Trainium2 NeuronCores.

Strategy: pure data parallel over the token dim. The LoRA update is folded
into the weight on the host (W' = W + 4.0 * B @ A, rank-8 update), so the
device kernel is a plain linear. x and W' are cast to bf16 on the host
(end-to-end rel fro err ~3e-3, well under the 2e-2 gate); out is written
bf16 and upcast on the host. PSUM accumulation stays fp32.

Per core: out[2048, 1024] = x_shard[2048, 1024] @ w' + bias, built as
256 matmuls: stationary x [128d,128tok] bf16 (FWL), moving w [128d,512o].

Key constraint discovered by tracing: all dma_starts share 8 DMAHW
completion-semaphore slots, and each completion costs the ~2-3.5us HBM
read+receipt latency, so descriptor N+8's issue stalls on descriptor N's
completion. The host therefore pre-swizzles x and w into layouts where
the whole startup working set is EXACTLY 8 descriptors, in consumption
order:
  xR[p, g*4096 + t*512 + nn] = x[g*512+nn, t*128+p]   (g=token group of
      512, t=contraction tile, nn=token within group, p=partition)
  wR[p, t*1024 + o]          = w'[t*128+p, o]
  sync queue:   x(t0,g0) | x(t1-3,g0) | x(t4-7,g0) | x(g1) | x(g2) | x(g3)
  scalar queue: w(t0h0) | w(t0h1,t1) | w(t2-4) | w(t5-7) | bias | 16 stores

Other structure:
  - ~15 warmup matmuls on zeroed bf16 scratch keep the PE busy from the
    engine preamble to first-data (~10.5us) so the HAM clock gate
    (1.2 -> 2.4 GHz after ~3.4us of sustained activity) flips early.
  - group 0 runs d-outer (arrival order); its last d-row goes i-outer
    with immediate per-tile eviction so psum slots free staggered.
  - groups 1-3 run i-outer / d-inner: evictions overlap accumulation.
  - the final tile goes h-outer so only one [128,512] eviction + 128KB
    store remain after the last matmul.
"""

import os
import sys

import numpy as np

for _p in ("/opt/trn_rl_repo", "/opt/pypackages"):
    if os.path.isdir(_p) and _p not in sys.path:
        sys.path.append(_p)

try:
    import jax

    jax.config.update(
        "jax_compilation_cache_dir", os.path.expanduser("~/.cache/jax_bass_cache")
    )
    jax.config.update("jax_persistent_cache_min_compile_time_secs", 0.0)
except Exception:
    pass

try:
    # bass_utils imports this when tracing is requested via BASS_TRACE; the
    # agent image ships a stub antenv without it. Register a no-op fallback
    # so a trace request degrades to "no trace" instead of crashing.
    from antenv import axon_hooks as _axon_hooks  # noqa: F401
except ImportError:
    import types as _types

    import antenv as _antenv

    _hooks = _types.ModuleType("antenv.axon_hooks")
    _hooks._hook = None
    _hooks.set_axon_ntff_profile_hook = lambda h: setattr(_hooks, "_hook", h)
    _hooks.get_axon_ntff_profile_hook = lambda: _hooks._hook
    sys.modules["antenv.axon_hooks"] = _hooks
    _antenv.axon_hooks = _hooks

import ml_dtypes  # noqa: E402

import concourse.bass as bass  # noqa: E402,F401
import concourse.mybir as mybir  # noqa: E402
import concourse.tile as tile  # noqa: E402
from concourse import bacc  # noqa: E402
from concourse.bass_utils import run_bass_kernel_spmd  # noqa: E402

N_CORES = 8
N_TOK, D_IN, D_OUT = 16384, 1024, 1024
N_SHARD = N_TOK // N_CORES  # 2048 tokens per core
P = 128
SCALING = 4.0  # alpha / r = 32 / 8
BF16 = ml_dtypes.bfloat16

KT = D_IN // P  # 8 contraction tiles
NBLK = 512  # tokens per group (4 psum tiles of 128)
GRP = NBLK // P  # 4 psum tiles accumulated concurrently (8 banks)
OH = 512  # one PSUM bank of fp32 = max moving free dim
NGRP = N_SHARD // NBLK
XCOL = KT * NBLK  # 4096 xR columns per token group
N_WARM = 14

_CACHE: dict = {}


def build_nc():
    f32 = mybir.dt.float32
    bf16 = mybir.dt.bfloat16
    nc = bacc.Bacc("TRN2", target_bir_lowering=False, debug=False)

    xR = nc.dram_tensor("xR", [P, NGRP * XCOL], bf16, kind="ExternalInput")
    wR = nc.dram_tensor("wR", [P, KT * D_OUT], bf16, kind="ExternalInput")
    bias = nc.dram_tensor("bias", [1, D_OUT], f32, kind="ExternalInput")
    out = nc.dram_tensor("out", [N_SHARD, D_OUT], bf16, kind="ExternalOutput")

    with tile.TileContext(nc) as tc:
        with tc.tile_pool(name="const", bufs=1) as const_pool, \
                tc.tile_pool(name="op", bufs=6) as out_pool, \
                tc.tile_pool(name="ps", bufs=GRP, space="PSUM") as psum_pool:
            x_all = const_pool.tile([P, NGRP * XCOL], bf16, name="x_all")
            w_all = const_pool.tile([P, KT * D_OUT], bf16, name="w_all")
            bias_sb = const_pool.tile([P, D_OUT], f32)
            gate_sb = const_pool.tile([P, 1], bf16, name="gate")

            def xs(g, t, i):  # stationary slice for (group, d-tile, tok blk)
                c0 = g * XCOL + t * NBLK + i * P
                return x_all[:, c0:c0 + P]

            def ws(t, h):  # moving slice for (d-tile, o-half)
                c0 = t * D_OUT + h * OH
                return w_all[:, c0:c0 + OH]

            # Warmup scratch + matmuls (see module docstring).
            warm = const_pool.tile([P, 256], bf16)
            nc.gpsimd.memset(warm[:], 0.0)
            warm_ps = psum_pool.tile([P, 256], f32, name="warm_ps", tag="psum")
            for _ in range(N_WARM):
                nc.tensor.matmul(warm_ps[:], warm[:, 0:P], warm[:],
                                 start=True, stop=True)

            # Startup descriptors, SMALL and interleaved in strict need
            # order: the SDMA engines round-robin over all in-flight
            # descriptors at packet granularity, so a descriptor's
            # completion time ~ (bytes in flight ahead of it) / BW. Big
            # early descriptors destroy arrival ordering; bulk (groups 1-3
            # of x) goes last as 1MB descriptors on recycled slots.
            def wdma(c0, c1):
                nc.scalar.dma_start(w_all[:, c0:c1], wR[:, c0:c1])

            def xdma(c0, c1):
                nc.sync.dma_start(x_all[:, c0:c1], xR[:, c0:c1])

            wdma(0, OH)                      # w t0 h0
            xdma(0, NBLK)                    # x t0 g0
            wdma(OH, D_OUT)                  # w t0 h1
            xdma(NBLK, 2 * NBLK)             # x t1 g0
            wdma(D_OUT, 2 * D_OUT)           # w t1
            xdma(2 * NBLK, 3 * NBLK)         # x t2 g0
            wdma(2 * D_OUT, 3 * D_OUT)       # w t2
            xdma(3 * NBLK, 4 * NBLK)         # x t3 g0
            wdma(3 * D_OUT, 4 * D_OUT)       # w t3
            xdma(4 * NBLK, 6 * NBLK)         # x t4-t5 g0
            wdma(4 * D_OUT, 6 * D_OUT)       # w t4-t5
            xdma(6 * NBLK, 8 * NBLK)         # x t6-t7 g0
            wdma(6 * D_OUT, 8 * D_OUT)       # w t6-t7
            for g in range(1, NGRP):
                xdma(g * XCOL, (g + 1) * XCOL)

            # bias must add ZERO bytes to the w/x HWDGE rings (their byte
            # budget vs need-times has no slack — a 512KB broadcast ahead
            # of w t3 measured a 5us stall), and issuing it immediately on
            # the gpsimd SWDGE ring steals the critical first ~8us of SDMA
            # bandwidth (also measured). Gate it instead: a dummy copy
            # reading an x t3 column makes the SWDGE queue sit idle until
            # ~+8us, then the broadcast lands ~+10us — well before the
            # first eviction needs it (~+16us).
            nc.gpsimd.tensor_copy(gate_sb[:], x_all[:, 3 * NBLK:3 * NBLK + 1])
            nc.gpsimd.dma_start(bias_sb[:], bias[:].to_broadcast((P, D_OUT)))

            def evict(g, i, psum):
                n0 = g * NBLK + i * P
                o_sb = out_pool.tile([P, D_OUT], bf16)
                for h in range(2):
                    sl = slice(h * OH, (h + 1) * OH)
                    nc.vector.tensor_add(o_sb[:, sl], psum[:, sl],
                                         bias_sb[:, sl])
                nc.scalar.dma_start(out[n0:n0 + P, :], o_sb[:])

            # Group 0: d-outer / h-middle / i-inner for t0-t5 (arrival
            # order), then the last TWO d rows go i-outer with immediate
            # per-tile eviction: TT(g0,i0) starts ~3.5us before group end,
            # so all four evictions are done when group 1 reuses the psum
            # slots (a one-row stagger left a ~1us reuse stall).
            psums = [
                psum_pool.tile([P, D_OUT], f32, name=f"ps_g0_{i}", tag="psum")
                for i in range(GRP)
            ]
            for t in range(KT - 2):
                for h in range(2):
                    osl = slice(h * OH, (h + 1) * OH)
                    for i in range(GRP):
                        nc.tensor.matmul(psums[i][:, osl], xs(0, t, i),
                                         ws(t, h), start=(t == 0), stop=False)
            for i in range(GRP):
                for t in (KT - 2, KT - 1):
                    for h in range(2):
                        osl = slice(h * OH, (h + 1) * OH)
                        nc.tensor.matmul(psums[i][:, osl], xs(0, t, i),
                                         ws(t, h), start=False,
                                         stop=(t == KT - 1))
                evict(0, i, psums[i])

            # Groups 1-3: i-outer / d-inner; final tile h-outer.
            for g in range(1, NGRP):
                for i in range(GRP):
                    last = (g == NGRP - 1 and i == GRP - 1)
                    if not last:
                        psum = psum_pool.tile([P, D_OUT], f32,
                                              name=f"ps_g{g}_{i}", tag="psum")
                        for t in range(KT):
                            for h in range(2):
                                osl = slice(h * OH, (h + 1) * OH)
                                nc.tensor.matmul(psum[:, osl], xs(g, t, i),
                                                 ws(t, h), start=(t == 0),
                                                 stop=(t == KT - 1))
                        evict(g, i, psum)
                    else:
                        # Final tile: h-outer on TWO separate one-bank psum
                        # tiles (a single [128,1024] tile makes h1's
                        # start=True wait on h0's eviction — tile-granular
                        # WAR — costing ~1.2us on the critical tail).
                        n0 = g * NBLK + i * P
                        o_sb = out_pool.tile([P, D_OUT], bf16)
                        for h in range(2):
                            ps_h = psum_pool.tile([P, OH], f32,
                                                  name=f"ps_last_h{h}",
                                                  tag="psum")
                            osl = slice(h * OH, (h + 1) * OH)
                            for t in range(KT):
                                nc.tensor.matmul(ps_h[:], xs(g, t, i),
                                                 ws(t, h), start=(t == 0),
                                                 stop=(t == KT - 1))
                            nc.vector.tensor_add(o_sb[:, osl], ps_h[:],
                                                 bias_sb[:, osl])
                            nc.scalar.dma_start(out[n0:n0 + P, osl],
                                                o_sb[:, osl])

    nc.finalize()
    return nc


def _get_nc():
    if "nc" not in _CACHE:
        _CACHE["nc"] = build_nc()
    return _CACHE["nc"]


def _swizzle_x(x_shard):
    # [2048, 1024] -> xR[p, g*4096 + t*512 + nn] = x[g*512+nn, t*128+p]
    v = x_shard.reshape(NGRP, NBLK, KT, P)  # [g, nn, t, p]
    v = v.transpose(3, 0, 2, 1)  # [p, g, t, nn]
    return np.ascontiguousarray(v.reshape(P, NGRP * XCOL))


def kernel(x, weight, bias, A, B):
    x = np.asarray(x, dtype=np.float32)
    weight = np.asarray(weight, dtype=np.float32)
    bias = np.asarray(bias, dtype=np.float32)
    A = np.asarray(A, dtype=np.float32)
    B = np.asarray(B, dtype=np.float32)

    # Fold the rank-8 LoRA update into the weight (exact up to fp32 rounding).
    w_eff = (
        weight.astype(np.float64) + SCALING * (B.astype(np.float64) @ A.astype(np.float64))
    ).astype(np.float32)
    # wR[p, t*1024 + o] = w_eff.T[t*128+p, o] = w_eff[o, t*128+p]
    wR = np.ascontiguousarray(
        w_eff.T.astype(BF16).reshape(KT, P, D_OUT).transpose(1, 0, 2).reshape(
            P, KT * D_OUT)
    )
    xb = x.astype(BF16)
    bias2d = np.ascontiguousarray(bias.reshape(1, D_OUT))

    nc = _get_nc()
    in_maps = [
        {
            "xR": _swizzle_x(xb[c * N_SHARD:(c + 1) * N_SHARD]),
            "wR": wR,
            "bias": bias2d,
        }
        for c in range(N_CORES)
    ]
    trace_kwargs = {}
    if os.environ.get("KERNEL_TRACE") == "1":
        trace_kwargs = {"trace": True}
    res = run_bass_kernel_spmd(nc, in_maps, list(range(N_CORES)), **trace_kwargs)
    _CACHE["last_results"] = res
    out = np.concatenate([r["out"] for r in res.results], axis=0)
    return out.astype(np.float32)
